# revision 12
# baseline (speedup 1.0000x reference)
"""HardNegTripletMarginLoss on 8 Trainium2 NeuronCores (Bass/Tile).

Strategy (anchors row-sharded across 8 cores, embeddings replicated):
  - Host: normalize rows (as reference), stable-sort rows by label, and give
    each core a column-ROTATED copy of Xn^T quantized to fp8-e4m3 with a
    256-col halo so each 128-anchor block's same-label window sits at a
    statically known 640-col range.  One SPMD program serves all 8 cores.
  - Device, per 128-anchor block (8 blocks/core), the 8192 columns stream
    through PSUM as four [128,2048] generations (ring-2 over the 8 banks):
      * PE: fp8 DoubleRow matmuls (0.5 cyc/col) fill each generation; the
        K=128 contraction is laid out as [64 partitions x 2] fp8 pairs.
        lhsT carries -X so v = -2*g lands in PSUM.  A one-hot bf16 matmul
        lifts same-label cols of the diag window by +BIG.
      * DVE: exact min over cols [0,864) of each generation (lifted
        positives auto-excluded), plus one exact max over the 640-col
        lifted window per block (= BIG + hardest-positive distance; makes
        d_ap exact, no host-side sampling needed).
      * ACT: exp-accumulate softmin over cols [864,2048) with a CONSTANT
        bias -2T (v >= -2 for normalized rows), so no per-row softmin
        reference needs host sampling.  T=56 keeps the smallest terms
        ~e^-78, far above f32 underflow.
    The 864/1184 split balances DVE (1.04 ns/col + ~300ns/read) against
    ACT (0.83 ns/col + ~700ns/read incl. accumulator readback).
  - Host: d_ap = sqrt(s_i+1+max-BIG), d_an = sqrt(s_i+1+min(exact, softmin)),
    relu(d_ap-d_an+margin), AvgNonZero reduction; exact per-row repair for
    any underflowed soft sums (expected none at T=56).

This walrus build rejects instructions carrying >1 sync wait, so
Bass.to_json_bytes is wrapped to split multi-wait instructions into
single-wait Drain carriers on the same engine.
"""

import json
import os
import sys
import types
import ctypes

for _p in ("/opt/trn_rl_repo", "/root/.axon_site/_ro/trn_rl_repo"):
    if os.path.isdir(_p) and _p not in sys.path:
        sys.path.append(_p)

import numpy as np
import ml_dtypes
import concourse.bass as bass
import concourse.tile as tile
from concourse import mybir
from concourse.bass_utils import run_bass_kernel_spmd
from contextlib import ExitStack

P = 128
N = 8192
D = 128
NCORES = 8
M = N // NCORES            # anchors per core
NBLK = M // P              # anchor blocks per core
HALO = 256                 # halo cols so every window is contiguous
XW = N + HALO              # xt width
GEN = 2048                 # PSUM generation width (4 banks)
DW = 864                   # DVE exact-min cols per generation
WIN = 640                  # lifted same-label window per block
BIG = 16.0
MARGIN = 0.05
TSOFT = 56.0               # softmin temperature (in d^2 units)
F32 = mybir.dt.float32
BF16 = mybir.dt.bfloat16
FP8 = mybir.dt.float8e4
BF16NP = ml_dtypes.bfloat16
FP8NP = ml_dtypes.float8_e4m3

# output layout: [0,32) DVE mins (4/block), [32,64) ACT soft sums (4/block),
# [64,72) DVE window max (1/block)
OUT_W = 9 * NBLK

LAST_RESULTS = None        # BassKernelResults of the most recent run (for test.py)


def _install_wait_split_patch():
    if getattr(bass.Bass, "_wait_split_patched", False):
        return
    orig = bass.Bass.to_json_bytes

    def patched(self):
        raw = orig(self)
        d = json.loads(raw)
        changed = False
        for fn in d.get("functions", []):
            for blk in fn.get("blocks", []):
                out, k = [], 0
                for ins in blk.get("instructions", []):
                    si = ins.get("sync_info") or {}
                    waits = si.get("on_wait") or []
                    if len(waits) > 1:
                        changed = True
                        for w in waits[:-1]:
                            k += 1
                            out.append({
                                "name": f"{ins['name']}-sw{k}",
                                "opcode": "Drain",
                                "engine": ins["engine"],
                                "ins": [],
                                "outs": [],
                                "is_reset_sema": False,
                                "debug": ins.get("debug", 0),
                                "sync_info": {"on_wait": [w], "on_update": []},
                            })
                        si["on_wait"] = [waits[-1]]
                    out.append(ins)
                blk["instructions"] = out
        return json.dumps(d).encode() if changed else raw

    bass.Bass.to_json_bytes = patched
    bass.Bass._wait_split_patched = True


def _ensure_ntff_hook():
    """Best-effort: restore the axon NTFF profile hook this image dropped."""
    if "antenv.axon_hooks" in sys.modules:
        return
    try:
        lib = ctypes.CDLL("/opt/axon/libaxon_pjrt.so")
        if not hasattr(lib, "axon_start_nrt_profile"):
            return
        from trn_agent_boot.trn_boot import _ntff_profile_via_ctypes
        hook = _ntff_profile_via_ctypes("/opt/axon/libaxon_pjrt.so")
        mod = types.ModuleType("antenv.axon_hooks")
        mod._hook = hook
        mod.get_axon_ntff_profile_hook = lambda: mod._hook
        mod.set_axon_ntff_profile_hook = lambda h: setattr(mod, "_hook", h)
        sys.modules["antenv.axon_hooks"] = mod
        import antenv
        antenv.axon_hooks = mod
    except Exception:
        pass


def _fill_pieces(l, h, k):
    """Fill pieces for stream tile k of half-gen h of block l.

    Returns list of (psum_off, halo_off, width).  The 512-col stream tile
    maps to halo col l*128 + h*2048 + k*512; tiles whose rotated range
    crosses N wrap to halo col-(N) and split into two pieces.
    """
    ps_off = k * 512
    o = l * P + h * GEN + k * 512
    # rotated start of this tile (halo col o corresponds to rotated o-HALO)
    r0 = o - HALO
    if r0 >= N:
        return [(ps_off, o - N, 512)]
    if r0 + 512 <= N:
        return [(ps_off, o, 512)]
    # split at rotated N: first piece [r0, N), second wraps to rotated 0
    w1 = N - r0
    return [(ps_off, o, w1), (ps_off + w1, HALO, 512 - w1)]


def _build_nc():
    nc = bass.Bass("TRN2", target_bir_lowering=False, debug=False)
    xt_d = nc.dram_tensor("xt", [P, XW], FP8, kind="ExternalInput")
    xa_d = nc.dram_tensor("xa", [P, 2, M], FP8, kind="ExternalInput")
    ohc_d = nc.dram_tensor("ohc", [64, 2048], BF16, kind="ExternalInput")
    oha_d = nc.dram_tensor("oha", [64, M], BF16, kind="ExternalInput")
    out_d = nc.dram_tensor("per_out", [P, OUT_W], F32, kind="ExternalOutput")

    DR = mybir.MatmulPerfMode.DoubleRow

    with tile.TileContext(nc) as tc, ExitStack() as ctx:
        inpool = ctx.enter_context(tc.tile_pool(name="ins", bufs=1))
        ppool = ctx.enter_context(tc.tile_pool(name="psum", bufs=2, space="PSUM"))
        spool = ctx.enter_context(tc.tile_pool(name="scr", bufs=2))
        apool = ctx.enter_context(tc.tile_pool(name="acc", bufs=1))
        fpool = ctx.enter_context(tc.tile_pool(name="fin", bufs=1))

        # preload the exp activation table during the input DMAs so the first
        # soft generation doesn't pay the ~1.3us table load
        warm = fpool.tile([P, 1], F32, tag="warm")
        nc.vector.memset(warm[:], 0.0)
        nc.scalar.activation(warm[:], warm[:], mybir.ActivationFunctionType.Exp)
        cbias = fpool.tile([P, 1], F32, tag="cbias", name="cbias")
        nc.vector.memset(cbias[:], -2.0 * TSOFT)

        # inputs; issue order on each queue = arrival order.
        xa = inpool.tile([P, 2, M], FP8, tag="xa")
        nc.sync.dma_start(xa[:], xa_d.ap()[:, :, :])
        xt = inpool.tile([P, XW], FP8, tag="xt")
        nc.sync.dma_start(xt[:, 0:1024], xt_d.ap()[:, 0:1024])
        oha = inpool.tile([64, M], BF16, tag="oha")
        nc.sync.dma_start(oha[:], oha_d.ap()[:, :])
        ohc = inpool.tile([64, 2048], BF16, tag="ohc")
        nc.sync.dma_start(ohc[:], ohc_d.ap()[:, :])
        # bulk xt chunks from the (otherwise idle) gpsimd queue
        for a in range(1024, XW, 1024):
            b = min(a + 1024, XW)
            nc.gpsimd.dma_start(xt[:, a:b], xt_d.ap()[:, a:b])

        accm = apool.tile([P, 5 * NBLK], F32, tag="accm", name="accm")  # DVE
        accs = apool.tile([P, 4 * NBLK], F32, tag="accs", name="accs")  # ACT

        for l in range(NBLK):
            lhsT = xa[:, :, l * P:(l + 1) * P]
            for h in range(4):
                ps = ppool.tile([P, GEN], F32, tag="ps")
                for k in range(4):
                    pieces = _fill_pieces(l, h, k)
                    lifted = (h == 0 and k < 2)
                    for pi, (ps_off, o, w) in enumerate(pieces):
                        rhs = xt[:, o:o + w].unsqueeze(1).broadcast_to([P, 2, w])
                        nc.tensor.matmul(
                            ps[:, ps_off:ps_off + w],
                            lhsT=lhsT, rhs=rhs,
                            start=(pi == 0),
                            stop=(pi == len(pieces) - 1) and not lifted,
                            perf_mode=DR)
                    if lifted:
                        # one-hot lift of the same-label window (+BIG)
                        wl = 512 if k == 0 else WIN - 512
                        nc.tensor.matmul(
                            ps[:, k * 512:k * 512 + wl],
                            lhsT=oha[:, l * P:(l + 1) * P],
                            rhs=ohc[:, l * P + k * 512:l * P + k * 512 + wl],
                            start=False, stop=True, skip_group_check=True)
                if h == 0:
                    # exact hardest-positive: max over the lifted window
                    nc.vector.tensor_reduce(
                        accm[:, 32 + l:33 + l], ps[:, 0:WIN],
                        op=mybir.AluOpType.max, axis=mybir.AxisListType.X)
                nc.vector.tensor_reduce(
                    accm[:, 4 * l + h:4 * l + h + 1], ps[:, 0:DW],
                    op=mybir.AluOpType.min, axis=mybir.AxisListType.X)
                sb = spool.tile([P, GEN - DW], BF16, tag="sb")
                nc.scalar.activation(
                    sb[:], ps[:, DW:GEN], mybir.ActivationFunctionType.Exp,
                    bias=cbias[:, 0:1], scale=-TSOFT,
                    accum_out=accs[:, 4 * l + h:4 * l + h + 1])

        nc.sync.dma_start(out_d.ap()[:, 0:5 * NBLK], accm[:])
        nc.sync.dma_start(out_d.ap()[:, 5 * NBLK:9 * NBLK], accs[:])
    return nc


def _reference_fallback(embeddings, labels):
    x = embeddings / np.maximum(
        np.sqrt((embeddings * embeddings).sum(1, keepdims=True)), 1e-12)
    sq = (x * x).sum(1)
    d2 = sq[:, None] + sq[None, :] - 2.0 * (x @ x.T)
    dist = np.sqrt(np.maximum(d2, 0.0))
    same = labels[:, None] == labels[None, :]
    eye = np.eye(len(labels), dtype=bool)
    pos, neg = same & ~eye, ~same
    d_ap = np.where(pos, dist, -np.inf).max(1)
    d_an = np.where(neg, dist, np.inf).min(1)
    valid = pos.any(1) & neg.any(1)
    per = np.maximum(d_ap - d_an + MARGIN, 0.0)
    per = np.where(valid, per, 0.0)
    nz = (per > 0).sum()
    return np.float32(per.sum() / max(nz, 1)) if nz > 0 else np.float32(0.0)


def kernel(embeddings: np.ndarray, labels: np.ndarray) -> np.ndarray:
    global LAST_RESULTS
    emb = np.asarray(embeddings, dtype=np.float32)
    lab = np.asarray(labels).reshape(-1)

    counts = np.bincount(lab.astype(np.int64) - lab.min())
    if emb.shape != (N, D) or counts.max() > 256 or len(np.unique(lab)) < 2:
        return np.array(_reference_fallback(emb, lab), dtype=np.float32)

    norms = np.sqrt((emb * emb).sum(1, keepdims=True, dtype=np.float32))
    xn = emb / np.maximum(norms, np.float32(1e-12))
    s = (xn * xn).sum(1, dtype=np.float32)

    perm = np.argsort(lab, kind="stable")
    xs = xn[perm]
    ls = lab[perm]
    ss = s[perm]

    uniq = np.unique(ls)
    code = np.searchsorted(uniq, ls).astype(np.int64)
    assert len(uniq) <= 64

    # fp8 operand planes (device sees these exact values)
    xs8 = xs.astype(FP8NP)          # rhs plane
    xa8 = (-2.0 * xs).astype(FP8NP)  # lhsT slot-0 plane (slot 1 is zero)
    xs8f = xs8.astype(np.float32)
    xa8f = xa8.astype(np.float32)

    _install_wait_split_patch()
    _ensure_ntff_hook()
    nc = _build_nc()

    in_maps = []
    for c in range(NCORES):
        lo = c * M
        rot = np.roll(np.arange(N), -lo)
        halo_idx = rot[(np.arange(XW) - HALO) % N]
        xt = np.ascontiguousarray(xs8[halo_idx].T)
        xa = np.zeros((P, 2, M), dtype=FP8NP)
        xa[:, 0, :] = xa8[lo:lo + M].T
        ohc = (code[halo_idx[:2048]][None, :] == np.arange(64)[:, None]).astype(BF16NP)
        oha = (BIG * (code[rot[0:M]][None, :] == np.arange(64)[:, None])).astype(BF16NP)
        in_maps.append({"xt": xt, "xa": xa, "ohc": ohc, "oha": oha})

    res = run_bass_kernel_spmd(nc, in_maps, core_ids=list(range(NCORES)))
    LAST_RESULTS = res

    d_ap_all = np.empty(N, dtype=np.float64)
    d_an_all = np.empty(N, dtype=np.float64)
    bad = []
    for c in range(NCORES):
        o = np.asarray(res.results[c]["per_out"], dtype=np.float64)
        lo = c * M
        rot = np.roll(np.arange(N), -lo)
        for l in range(NBLK):
            rows = rot[np.arange(l * P, (l + 1) * P)]
            m_exact = o[:, 4 * l:4 * l + 4].min(1)
            ssum = o[:, 5 * NBLK + 4 * l:5 * NBLK + 4 * l + 4].sum(1)
            mxl = o[:, 32 + l]
            ok = np.isfinite(ssum) & (ssum > 0) & np.isfinite(m_exact) & np.isfinite(mxl)
            bad.extend(rows[~ok])
            mn_soft = np.where(ssum > 0,
                               -2.0 - np.log(np.maximum(ssum, 1e-45)) / TSOFT,
                               np.inf)
            mn = np.minimum(m_exact, mn_soft)
            s_i = ss[rot[np.arange(l * P, (l + 1) * P)]]
            d_an_all[rows] = np.sqrt(np.maximum(s_i + 1.0 + mn, 0.0))
            d_ap_all[rows] = np.sqrt(np.maximum(s_i + 1.0 + (mxl - BIG), 0.0))

    if bad:
        # underflowed/degenerate anchors: recompute exactly on host (rare)
        for i in bad:
            v = xs8f @ xa8f[i]  # = -2 g with device quantization
            d2 = ss[i] + 1.0 + v
            d = np.sqrt(np.maximum(d2, 0.0))
            samel = ls == ls[i]
            posm = samel.copy()
            posm[i] = False
            d_ap_all[i] = d[posm].max() if posm.any() else 0.0
            d_an_all[i] = d[~samel].min()

    # map sorted-order d_ap/d_an back: rows above are sorted indices
    per = np.maximum(d_ap_all - d_an_all + MARGIN, 0.0)
    nz = int((per > 0).sum())
    if nz == 0:
        return np.array(0.0, dtype=np.float32)
    return np.array(np.float32(per.sum() / nz), dtype=np.float32)


if __name__ == "__main__":
    from concourse import bass_utils
    import tempfile
    _install_wait_split_patch()
    nc = _build_nc()
    td = tempfile.mkdtemp(prefix="tripletk_")
    print(bass_utils.compile_bass_kernel(nc, td))


# revision 17
# speedup vs baseline: 1.0719x; 1.0719x over previous
"""HardNegTripletMarginLoss on 8 Trainium2 NeuronCores (Bass/Tile).

Strategy (anchors row-sharded across 8 cores, embeddings replicated):
  - Host: normalize rows (as reference), stable-sort rows by label, and give
    each core a column-ROTATED copy of Xn^T quantized to fp8-e4m3 with a
    256-col halo so each 128-anchor block's same-label window sits at a
    statically known 640-col range.  One SPMD program serves all 8 cores.
  - Device, per 128-anchor block (8 blocks/core), the 8192 columns stream
    through PSUM as four [128,2048] generations (ring-2 over the 8 banks):
      * PE: fp8 DoubleRow matmuls (0.5 cyc/col) fill each generation; the
        K=128 contraction is laid out as [64 partitions x 2] fp8 pairs.
        lhsT carries -X so v = -2*g lands in PSUM.  A one-hot bf16 matmul
        lifts same-label cols of the diag window by +BIG.
      * DVE: exact min over cols [0,864) of each generation (lifted
        positives auto-excluded), plus one exact max over the 640-col
        lifted window per block (= BIG + hardest-positive distance; makes
        d_ap exact, no host-side sampling needed).
      * ACT: exp-accumulate softmin over cols [864,2048) with a CONSTANT
        bias -2T (v >= -2 for normalized rows), so no per-row softmin
        reference needs host sampling.  T=56 keeps the smallest terms
        ~e^-78, far above f32 underflow.
    The 864/1184 split balances DVE (1.04 ns/col + ~300ns/read) against
    ACT (0.83 ns/col + ~700ns/read incl. accumulator readback).
  - Host: d_ap = sqrt(s_i+1+max-BIG), d_an = sqrt(s_i+1+min(exact, softmin)),
    relu(d_ap-d_an+margin), AvgNonZero reduction; exact per-row repair for
    any underflowed soft sums (expected none at T=56).

This walrus build rejects instructions carrying >1 sync wait, so
Bass.to_json_bytes is wrapped to split multi-wait instructions into
single-wait Drain carriers on the same engine.
"""

import json
import os
import sys
import types
import ctypes

for _p in ("/opt/trn_rl_repo", "/root/.axon_site/_ro/trn_rl_repo"):
    if os.path.isdir(_p) and _p not in sys.path:
        sys.path.append(_p)

import numpy as np
import ml_dtypes
import concourse.bass as bass
import concourse.tile as tile
from concourse import mybir
from concourse.bass_utils import run_bass_kernel_spmd
from contextlib import ExitStack

P = 128
N = 8192
D = 128
NCORES = 8
M = N // NCORES            # anchors per core
NBLK = M // P              # anchor blocks per core
HALO = 256                 # halo cols so every window is contiguous
XW = N + HALO              # xt width
GEN = 2048                 # PSUM generation width (4 banks)
DW = 960                   # DVE exact-min cols per generation
WIN = 640                  # lifted same-label window per block
BIG = 16.0
MARGIN = 0.05
TSOFT = 56.0               # softmin temperature (in d^2 units)
F32 = mybir.dt.float32
BF16 = mybir.dt.bfloat16
FP8 = mybir.dt.float8e4
BF16NP = ml_dtypes.bfloat16
FP8NP = ml_dtypes.float8_e4m3

# output layout: [0,32) DVE mins (4/block), [32,64) ACT soft sums (4/block),
# [64,72) DVE window max (1/block)
OUT_W = 9 * NBLK

LAST_RESULTS = None        # BassKernelResults of the most recent run (for test.py)


def _install_wait_split_patch():
    if getattr(bass.Bass, "_wait_split_patched", False):
        return
    orig = bass.Bass.to_json_bytes

    def patched(self):
        raw = orig(self)
        d = json.loads(raw)
        changed = False
        for fn in d.get("functions", []):
            for blk in fn.get("blocks", []):
                out, k = [], 0
                for ins in blk.get("instructions", []):
                    si = ins.get("sync_info") or {}
                    waits = si.get("on_wait") or []
                    if len(waits) > 1:
                        changed = True
                        for w in waits[:-1]:
                            k += 1
                            out.append({
                                "name": f"{ins['name']}-sw{k}",
                                "opcode": "Drain",
                                "engine": ins["engine"],
                                "ins": [],
                                "outs": [],
                                "is_reset_sema": False,
                                "debug": ins.get("debug", 0),
                                "sync_info": {"on_wait": [w], "on_update": []},
                            })
                        si["on_wait"] = [waits[-1]]
                    out.append(ins)
                blk["instructions"] = out
        return json.dumps(d).encode() if changed else raw

    bass.Bass.to_json_bytes = patched
    bass.Bass._wait_split_patched = True


def _ensure_ntff_hook():
    """Best-effort: restore the axon NTFF profile hook this image dropped."""
    if "antenv.axon_hooks" in sys.modules:
        return
    try:
        lib = ctypes.CDLL("/opt/axon/libaxon_pjrt.so")
        if not hasattr(lib, "axon_start_nrt_profile"):
            return
        from trn_agent_boot.trn_boot import _ntff_profile_via_ctypes
        hook = _ntff_profile_via_ctypes("/opt/axon/libaxon_pjrt.so")
        mod = types.ModuleType("antenv.axon_hooks")
        mod._hook = hook
        mod.get_axon_ntff_profile_hook = lambda: mod._hook
        mod.set_axon_ntff_profile_hook = lambda h: setattr(mod, "_hook", h)
        sys.modules["antenv.axon_hooks"] = mod
        import antenv
        antenv.axon_hooks = mod
    except Exception:
        pass


def _fill_pieces(l, h, k):
    """Fill pieces for stream tile k of half-gen h of block l.

    Returns list of (psum_off, halo_off, width).  The 512-col stream tile
    maps to halo col l*128 + h*2048 + k*512; tiles whose rotated range
    crosses N wrap to halo col-(N) and split into two pieces.
    """
    ps_off = k * 512
    o = l * P + h * GEN + k * 512
    # rotated start of this tile (halo col o corresponds to rotated o-HALO)
    r0 = o - HALO
    if r0 >= N:
        return [(ps_off, o - N, 512)]
    if r0 + 512 <= N:
        return [(ps_off, o, 512)]
    # split at rotated N: first piece [r0, N), second wraps to rotated 0
    w1 = N - r0
    return [(ps_off, o, w1), (ps_off + w1, HALO, 512 - w1)]


def _build_nc():
    nc = bass.Bass("TRN2", target_bir_lowering=False, debug=False)
    xt_d = nc.dram_tensor("xt", [P, XW], FP8, kind="ExternalInput")
    xa_d = nc.dram_tensor("xa", [P, M], FP8, kind="ExternalInput")
    ohc_d = nc.dram_tensor("ohc", [64, 2048], BF16, kind="ExternalInput")
    oha_d = nc.dram_tensor("oha", [64, M], BF16, kind="ExternalInput")
    out_d = nc.dram_tensor("per_out", [P, OUT_W], F32, kind="ExternalOutput")

    DR = mybir.MatmulPerfMode.DoubleRow

    with tile.TileContext(nc) as tc, ExitStack() as ctx:
        inpool = ctx.enter_context(tc.tile_pool(name="ins", bufs=1))
        ppool = ctx.enter_context(tc.tile_pool(name="psum", bufs=2, space="PSUM"))
        spool = ctx.enter_context(tc.tile_pool(name="scr", bufs=2))
        apool = ctx.enter_context(tc.tile_pool(name="acc", bufs=1))
        fpool = ctx.enter_context(tc.tile_pool(name="fin", bufs=1))

        # preload the exp activation table during the input DMAs so the first
        # soft generation doesn't pay the ~1.3us table load
        warm = fpool.tile([P, 1], F32, tag="warm")
        nc.vector.memset(warm[:], 0.0)
        nc.scalar.activation(warm[:], warm[:], mybir.ActivationFunctionType.Exp)
        cbias = fpool.tile([P, 1], F32, tag="cbias", name="cbias")
        nc.vector.memset(cbias[:], -2.0 * TSOFT)

        # inputs; issue order on each queue = arrival order.
        xa = inpool.tile([P, M], FP8, tag="xa")
        nc.sync.dma_start(xa[:], xa_d.ap()[:, :])
        xt = inpool.tile([P, XW], FP8, tag="xt")
        nc.sync.dma_start(xt[:, 0:1024], xt_d.ap()[:, 0:1024])
        oha = inpool.tile([64, M], BF16, tag="oha")
        nc.sync.dma_start(oha[:], oha_d.ap()[:, :])
        ohc = inpool.tile([64, 2048], BF16, tag="ohc")
        nc.sync.dma_start(ohc[:], ohc_d.ap()[:, :])
        # bulk xt chunks from the (otherwise idle) gpsimd queue
        for a, b in ((1024, 2048), (2048, 4096), (4096, 6144), (6144, XW)):
            nc.gpsimd.dma_start(xt[:, a:b], xt_d.ap()[:, a:b])

        accm = apool.tile([P, 5 * NBLK], F32, tag="accm", name="accm")  # DVE
        accs = apool.tile([P, 4 * NBLK], F32, tag="accs", name="accs")  # ACT

        for l in range(NBLK):
            lhsT = xa[:, l * P:(l + 1) * P]
            for h in range(4):
                ps = ppool.tile([P, GEN], F32, tag="ps")
                for k in range(4):
                    pieces = _fill_pieces(l, h, k)
                    lifted = (h == 0 and k < 2)
                    for pi, (ps_off, o, w) in enumerate(pieces):
                        nc.tensor.matmul(
                            ps[:, ps_off:ps_off + w],
                            lhsT=lhsT, rhs=xt[:, o:o + w],
                            start=(pi == 0),
                            stop=(pi == len(pieces) - 1) and not lifted)
                if h == 0:
                    # one-hot lift of the same-label window (+BIG); after all
                    # sweep fills so the PE streams while oha/ohc arrive
                    for k in (0, 1):
                        wl = 512 if k == 0 else WIN - 512
                        nc.tensor.matmul(
                            ps[:, k * 512:k * 512 + wl],
                            lhsT=oha[:, l * P:(l + 1) * P],
                            rhs=ohc[:, l * P + k * 512:l * P + k * 512 + wl],
                            start=False, stop=True, skip_group_check=True)
                if h == 0:
                    # exact hardest-positive: max over the lifted window
                    nc.vector.tensor_reduce(
                        accm[:, 32 + l:33 + l], ps[:, 0:WIN],
                        op=mybir.AluOpType.max, axis=mybir.AxisListType.X)
                nc.vector.tensor_reduce(
                    accm[:, 4 * l + h:4 * l + h + 1], ps[:, 0:DW],
                    op=mybir.AluOpType.min, axis=mybir.AxisListType.X)
                sb = spool.tile([P, GEN - DW], BF16, tag="sb")
                nc.scalar.activation(
                    sb[:], ps[:, DW:GEN], mybir.ActivationFunctionType.Exp,
                    bias=cbias[:, 0:1], scale=-TSOFT,
                    accum_out=accs[:, 4 * l + h:4 * l + h + 1])

        nc.sync.dma_start(out_d.ap()[:, 0:5 * NBLK], accm[:])
        nc.sync.dma_start(out_d.ap()[:, 5 * NBLK:9 * NBLK], accs[:])
    return nc


def _reference_fallback(embeddings, labels):
    x = embeddings / np.maximum(
        np.sqrt((embeddings * embeddings).sum(1, keepdims=True)), 1e-12)
    sq = (x * x).sum(1)
    d2 = sq[:, None] + sq[None, :] - 2.0 * (x @ x.T)
    dist = np.sqrt(np.maximum(d2, 0.0))
    same = labels[:, None] == labels[None, :]
    eye = np.eye(len(labels), dtype=bool)
    pos, neg = same & ~eye, ~same
    d_ap = np.where(pos, dist, -np.inf).max(1)
    d_an = np.where(neg, dist, np.inf).min(1)
    valid = pos.any(1) & neg.any(1)
    per = np.maximum(d_ap - d_an + MARGIN, 0.0)
    per = np.where(valid, per, 0.0)
    nz = (per > 0).sum()
    return np.float32(per.sum() / max(nz, 1)) if nz > 0 else np.float32(0.0)


def kernel(embeddings: np.ndarray, labels: np.ndarray) -> np.ndarray:
    global LAST_RESULTS
    emb = np.asarray(embeddings, dtype=np.float32)
    lab = np.asarray(labels).reshape(-1)

    counts = np.bincount(lab.astype(np.int64) - lab.min())
    if emb.shape != (N, D) or counts.max() > 256 or len(np.unique(lab)) < 2:
        return np.array(_reference_fallback(emb, lab), dtype=np.float32)

    norms = np.sqrt((emb * emb).sum(1, keepdims=True, dtype=np.float32))
    xn = emb / np.maximum(norms, np.float32(1e-12))
    s = (xn * xn).sum(1, dtype=np.float32)

    perm = np.argsort(lab, kind="stable")
    xs = xn[perm]
    ls = lab[perm]
    ss = s[perm]

    uniq = np.unique(ls)
    code = np.searchsorted(uniq, ls).astype(np.int64)
    assert len(uniq) <= 64

    # fp8 operand planes (device sees these exact values)
    xs8 = xs.astype(FP8NP)          # rhs plane
    xa8 = (-2.0 * xs).astype(FP8NP)  # lhsT slot-0 plane (slot 1 is zero)
    xs8f = xs8.astype(np.float32)
    xa8f = xa8.astype(np.float32)

    _install_wait_split_patch()
    _ensure_ntff_hook()
    nc = _build_nc()

    in_maps = []
    for c in range(NCORES):
        lo = c * M
        rot = np.roll(np.arange(N), -lo)
        halo_idx = rot[(np.arange(XW) - HALO) % N]
        xt = np.ascontiguousarray(xs8[halo_idx].T)
        xa = np.ascontiguousarray(xa8[lo:lo + M].T)
        ohc = (code[halo_idx[:2048]][None, :] == np.arange(64)[:, None]).astype(BF16NP)
        oha = (BIG * (code[rot[0:M]][None, :] == np.arange(64)[:, None])).astype(BF16NP)
        in_maps.append({"xt": xt, "xa": xa, "ohc": ohc, "oha": oha})

    res = run_bass_kernel_spmd(nc, in_maps, core_ids=list(range(NCORES)))
    LAST_RESULTS = res

    d_ap_all = np.empty(N, dtype=np.float64)
    d_an_all = np.empty(N, dtype=np.float64)
    bad = []
    for c in range(NCORES):
        o = np.asarray(res.results[c]["per_out"], dtype=np.float64)
        lo = c * M
        rot = np.roll(np.arange(N), -lo)
        for l in range(NBLK):
            rows = rot[np.arange(l * P, (l + 1) * P)]
            m_exact = o[:, 4 * l:4 * l + 4].min(1)
            ssum = o[:, 5 * NBLK + 4 * l:5 * NBLK + 4 * l + 4].sum(1)
            mxl = o[:, 32 + l]
            ok = np.isfinite(ssum) & (ssum > 0) & np.isfinite(m_exact) & np.isfinite(mxl)
            bad.extend(rows[~ok])
            mn_soft = np.where(ssum > 0,
                               -2.0 - np.log(np.maximum(ssum, 1e-45)) / TSOFT,
                               np.inf)
            mn = np.minimum(m_exact, mn_soft)
            s_i = ss[rot[np.arange(l * P, (l + 1) * P)]]
            d_an_all[rows] = np.sqrt(np.maximum(s_i + 1.0 + mn, 0.0))
            d_ap_all[rows] = np.sqrt(np.maximum(s_i + 1.0 + (mxl - BIG), 0.0))

    if bad:
        # underflowed/degenerate anchors: recompute exactly on host (rare)
        for i in bad:
            v = xs8f @ xa8f[i]  # = -2 g with device quantization
            d2 = ss[i] + 1.0 + v
            d = np.sqrt(np.maximum(d2, 0.0))
            samel = ls == ls[i]
            posm = samel.copy()
            posm[i] = False
            d_ap_all[i] = d[posm].max() if posm.any() else 0.0
            d_an_all[i] = d[~samel].min()

    # map sorted-order d_ap/d_an back: rows above are sorted indices
    per = np.maximum(d_ap_all - d_an_all + MARGIN, 0.0)
    nz = int((per > 0).sum())
    if nz == 0:
        return np.array(0.0, dtype=np.float32)
    return np.array(np.float32(per.sum() / nz), dtype=np.float32)


if __name__ == "__main__":
    from concourse import bass_utils
    import tempfile
    _install_wait_split_patch()
    nc = _build_nc()
    td = tempfile.mkdtemp(prefix="tripletk_")
    print(bass_utils.compile_bass_kernel(nc, td))
